# revision 46
# baseline (speedup 1.0000x reference)
"""Radon transform (bilinear grid-sample + row-sum) on 8 TRN2 NeuronCores.

Angle wedges sharded across 8 cores (rep-pure wedges: identity frame for
|cos|>=|sin|, transposed frame otherwise). Per core, per angle: detector
rays are PAIRED (x = 16*xi + 2g + e); each pair is decomposed into 32
8-row blocks of the content region, and one GPSIMD ap_gather index per
(pair, block) fetches a 16-wide column window (hop-4 aligned, overlapping
slab storage) covering all bilinear taps of both rays in that block. The
16 channels of each Q7 core hold 8 row phases x 2 batches, so every
gathered lane is useful. Per chunk, DVE multiplies the windows by
precomputed tap weights for e=0, half-folds, and segment-reduces each ray;
the e=1 product is reduced per-ray on the Scalar engine via activation
accum_out (splitting the reduce across engines — the gather and DVE
2-port ops share the Q7 SBUF port slot, so DVE work is kept lean). A
TensorE sel-matmul sums the 128 partitions into (group, batch) sinogram
rows. All indices/weights are input-independent and precomputed on host.
"""
import math
import os
import sys
from contextlib import ExitStack

import numpy as np

sys.path.insert(0, "/opt/trn_rl_repo")

import ml_dtypes  # noqa: E402

BF16 = ml_dtypes.bfloat16

# ─── geometry constants (hardcoded for 256x256, 180 angles, batch 2) ───
N_ANGLES = 180
IMG_SIZE = 256
BATCH = 2
S = int(math.ceil(math.sqrt(2.0) * IMG_SIZE))  # 363
PB = (S - IMG_SIZE) // 2                       # 53
ROFF = 53         # slab row/col origin = content origin
HOP = 4           # window alignment granularity
D = 16            # window width (bf16 elems per gather block)
NH = 64           # hop positions per slab row
NJ = 32           # 8-row blocks covering the 256 content rows
NELEM = NJ * NH   # 2048 gather blocks per slab partition
NMAX = NJ         # block slots per ray-pair (j used directly)
NXI = 23          # ray-pairs per Q7 group (x = 16*xi + 2g + e)
NRAYS_G = 2 * NXI              # 46 ray columns per group (e-major)
NTOT = NXI * NMAX              # 736 indices per slot per group
SEG = NMAX * D                 # 512 elems reduced per (ray, e)
SLOTS = 23
CHUNK_NXI = [12, 11]           # ray-pairs per chunk
NCHUNK = len(CHUNK_NXI)

CORE_ANGLES = [
    list(range(0, 23)), list(range(23, 46)),
    list(range(135, 158)), list(range(158, 180)),
    list(range(46, 69)), list(range(69, 91)),
    list(range(91, 113)), list(range(113, 135)),
]
CORE_REP = [0, 0, 0, 0, 1, 1, 1, 1]


def _angle_taps(k):
    """Content-region bilinear taps in rep-frame coords.

    Returns rep, xs (detector ray), j (8-row block), phi (row phase),
    cc (slab col = col-ROFF), ws (f32 weight)."""
    th = np.float32(k) * np.float32(np.pi / N_ANGLES)
    c = np.cos(th, dtype=np.float32)
    s = np.sin(th, dtype=np.float32)
    lin = np.linspace(-1.0, 1.0, S, dtype=np.float32)
    gx = c * lin[None, :] + s * lin[:, None]
    gy = -s * lin[None, :] + c * lin[:, None]
    ix = (gx + np.float32(1)) * np.float32(0.5) * np.float32(S - 1)
    iy = (gy + np.float32(1)) * np.float32(0.5) * np.float32(S - 1)
    x0 = np.floor(ix).astype(np.int64)
    y0 = np.floor(iy).astype(np.int64)
    wx = (ix - x0).astype(np.float32)
    wy = (iy - y0).astype(np.float32)
    rep = 0 if abs(c) >= abs(s) else 1
    rows_l, cols_l, ws_l, xs_l = [], [], [], []
    for dy in (0, 1):
        for dx in (0, 1):
            r = y0 + dy
            q = x0 + dx
            w = (wy if dy else 1 - wy) * (wx if dx else 1 - wx)
            m = ((r >= PB) & (r < PB + IMG_SIZE)
                 & (q >= PB) & (q < PB + IMG_SIZE) & (w != 0))
            _, xx = np.nonzero(m)
            rows_l.append(r[m])
            cols_l.append(q[m])
            ws_l.append(w[m])
            xs_l.append(xx)
    rows = np.concatenate(rows_l)
    cols = np.concatenate(cols_l)
    ws = np.concatenate(ws_l)
    xs = np.concatenate(xs_l)
    if rep:
        rows, cols = cols, rows
    j = (rows - ROFF) // 8
    phi = (rows - ROFF) % 8
    cc = cols - ROFF
    return rep, xs, j, phi, cc, ws


def _plan_angle(k):
    """Pair layout: pair p2 = x//2 (g = p2%8, xi = p2//8), e = x%2.

    Returns idx [184, NJ] int16 block ids per pair, and
    wt [2, 184, NJ, 8, D] f32 weights (e-major)."""
    rep, xs, j, phi, cc, ws = _angle_taps(k)
    p2 = xs // 2
    e = xs % 2
    qmin = np.full((184, NJ), 10 ** 6, np.int64)
    np.minimum.at(qmin, (p2, j), cc)
    hq = np.clip(qmin // HOP, 0, NH - D // HOP)
    wt = np.zeros((2, 184, NJ, 8, D), np.float32)
    kk = cc - HOP * hq[p2, j]
    assert kk.min() >= 0 and kk.max() < D, (k, kk.min(), kk.max())
    np.add.at(wt, (e, p2, j, phi, kk), ws)
    present = np.zeros((184, NJ), bool)
    present[p2, j] = True
    idx = np.where(present, np.arange(NJ)[None, :] * NH + hq, 0)
    return rep, idx.astype(np.int16), wt


_PLAN_CACHE = {}


def _get_plan():
    if "plan" in _PLAN_CACHE:
        return _PLAN_CACHE["plan"]
    slot_w = SLOTS * 2 * NTOT * D
    core_idx = []
    core_w = []
    for ci in range(8):
        idx_blob = np.zeros((128, SLOTS * (NTOT // 16)), np.int16)
        w_blob = np.zeros((64, slot_w // SLOTS * SLOTS), np.float32)
        for si, k in enumerate(CORE_ANGLES[ci]):
            rep, idx, wt = _plan_angle(k)
            assert rep == CORE_REP[ci]
            # idx[p2, j] -> group g = p2%8, n = xi*NJ + j
            ig = idx.reshape(NXI, 8, NJ).transpose(1, 0, 2).reshape(8, NTOT)
            wrap = ig.reshape(8, NTOT // 16, 16)
            for g in range(8):
                idx_blob[16 * g:16 * g + 16,
                         si * (NTOT // 16):(si + 1) * (NTOT // 16)] = wrap[g].T
            # wt[e, p2, j, phi, k] -> row 8g+phi,
            # chunk-major cols: [chunk][e][xi_local][j][k]
            wg = (wt.reshape(2, NXI, 8, NJ, 8, D)
                  .transpose(2, 4, 0, 1, 3, 5))  # [g, phi, e, xi, j, k]
            base = si * 2 * NTOT * D
            xi0 = 0
            for nxi in CHUNK_NXI:
                sz = 2 * nxi * NJ * D
                blockw = (wg[:, :, :, xi0:xi0 + nxi]
                          .reshape(64, sz))
                w_blob[:, base:base + sz] = blockw
                base += sz
                xi0 += nxi
        core_idx.append(idx_blob)
        core_w.append(w_blob.astype(BF16))
    sel = np.zeros((128, 16), np.float32)
    for p in range(128):
        sel[p, 2 * (p // 16) + (p % 2)] = 1.0
    plan = dict(core_idx=core_idx, core_w=core_w, sel=sel)
    _PLAN_CACHE["plan"] = plan
    return plan


def _build_slab(image, rep):
    """[128, NELEM*D] bf16: channel p%16 = 2*phi+b holds hop-4 overlapping
    windows of content rows ROFF+8j+phi (replicated across the 8 groups)."""
    fr = np.zeros((BATCH, S, S), np.float32)
    fr[:, PB:PB + IMG_SIZE, PB:PB + IMG_SIZE] = image[:, 0]
    if rep:
        fr = np.ascontiguousarray(np.transpose(fr, (0, 2, 1)))
    out = np.zeros((16, NELEM * D), np.float32)
    span = HOP * (NH - 1) + D  # 268 cols
    for phi in range(8):
        rows = fr[:, ROFF + phi: ROFF + phi + 8 * NJ: 8, ROFF:ROFF + span]
        win = np.lib.stride_tricks.sliding_window_view(rows, D, axis=2)
        win = win[:, :, ::HOP, :]  # [B, NJ, NH, D]
        assert win.shape == (BATCH, NJ, NH, D)
        for b in range(BATCH):
            out[2 * phi + b] = win[b].reshape(-1)
    out16 = out.astype(BF16)
    return np.ascontiguousarray(np.broadcast_to(
        out16[None], (8, 16, NELEM * D)).reshape(128, NELEM * D))


_PROG_CACHE = {}


def _build_program():
    if "prog" in _PROG_CACHE:
        return _PROG_CACHE["prog"]
    import concourse.bass as bass
    import concourse.mybir as mybir
    from concourse import library_config

    nc = bass.Bass()
    slab_d = nc.declare_dram_parameter("slab", [128, NELEM * D],
                                       mybir.dt.bfloat16, isOutput=False)
    idx_d = nc.declare_dram_parameter("idx", [128, SLOTS * (NTOT // 16)],
                                      mybir.dt.int16, isOutput=False)
    w_d = nc.declare_dram_parameter("w", [64, SLOTS * 2 * NTOT * D],
                                    mybir.dt.bfloat16, isOutput=False)
    sel_d = nc.declare_dram_parameter("sel", [128, 16], mybir.dt.float32,
                                      isOutput=False)
    out_d = nc.declare_dram_parameter("out", [16, SLOTS * NRAYS_G],
                                      mybir.dt.float32, isOutput=True)

    ctx = ExitStack()
    with ctx:
        slab_t = ctx.enter_context(
            nc.sbuf_tensor([128, NELEM * D], mybir.dt.bfloat16))
        idx_t = ctx.enter_context(
            nc.sbuf_tensor([128, SLOTS * (NTOT // 16)], mybir.dt.int16))
        maxw = max(CHUNK_NXI) * NJ  # windows per chunk
        wt_ts = [ctx.enter_context(
            nc.sbuf_tensor(f"wt{i}", [128, 2 * maxw * D], mybir.dt.bfloat16))
            for i in range(2)]
        # one gather per slot; two slot-parity halves for pipelining
        win_t = ctx.enter_context(
            nc.sbuf_tensor("win", [128, 2 * NTOT * D], mybir.dt.bfloat16))
        prod_ts = [ctx.enter_context(
            nc.sbuf_tensor(f"prod{i}", [128, maxw * D], mybir.dt.bfloat16))
            for i in range(2)]
        fold_t = ctx.enter_context(
            nc.sbuf_tensor([128, maxw * (D // 2)], mybir.dt.bfloat16))
        asink_t = ctx.enter_context(
            nc.sbuf_tensor([128, NMAX * D], mybir.dt.bfloat16))
        red_ts = [ctx.enter_context(
            nc.sbuf_tensor(f"red{i}", [128, NRAYS_G], mybir.dt.float32))
            for i in range(2)]
        sel_t = ctx.enter_context(nc.sbuf_tensor([128, 16], mybir.dt.float32))
        vscr_t = ctx.enter_context(nc.sbuf_tensor([128, 2], mybir.dt.float32))
        ascr_t = ctx.enter_context(nc.sbuf_tensor([128, 2], mybir.dt.float32))
        ascr16_t = ctx.enter_context(nc.sbuf_tensor([16, 2], mybir.dt.float32))
        sino_t = ctx.enter_context(
            nc.sbuf_tensor("sino", [16, SLOTS * NRAYS_G], mybir.dt.float32))
        psum_ts = [ctx.enter_context(
            nc.psum_tensor(f"ps{i}", [16, NRAYS_G], mybir.dt.float32))
            for i in range(2)]
        s_in = ctx.enter_context(nc.semaphore("s_in"))
        s_dma = ctx.enter_context(nc.semaphore("s_dma"))
        s_g = ctx.enter_context(nc.semaphore("s_g"))
        s_v = ctx.enter_context(nc.semaphore("s_v"))
        s_a = ctx.enter_context(nc.semaphore("s_a"))
        s_mm = ctx.enter_context(nc.semaphore("s_mm"))
        s_od = ctx.enter_context(nc.semaphore("s_od"))
        block = ctx.enter_context(nc.Block())

        # chunk schedule: (slot, cidx, xi0, nxi, cn, idx col offset, w offset)
        chunks = []
        for si in range(SLOTS):
            xi0 = 0
            ow = si * 2 * NTOT * D
            for cidx, nxi in enumerate(CHUNK_NXI):
                cn = nxi * NMAX
                chunks.append(dict(
                    si=si, cidx=cidx, xi0=xi0, nxi=nxi, cn=cn,
                    o16=si * (NTOT // 16) + xi0 * NMAX // 16,
                    ow=ow))
                ow += 2 * cn * D
                xi0 += nxi

        @block.sync
        def _(sync):
            sync.dma_start(out=slab_t[:], in_=slab_d[:]).then_inc(s_in, 16)
            sync.dma_start(out=idx_t[:], in_=idx_d[:]).then_inc(s_in, 16)
            sync.dma_start(out=sel_t[:], in_=sel_d[:]).then_inc(s_in, 16)
            for n, ch in enumerate(chunks):
                if n > 1:
                    sync.wait_ge(s_v, n - 1)  # wt buffer consumed
                wsrc = (w_d[:, ch["ow"]:ch["ow"] + 2 * ch["cn"] * D]
                        .unsqueeze(1)
                        .broadcast_to([64, 2, 2 * ch["cn"] * D]))
                sync.dma_start(out=wt_ts[n % 2][:, :2 * ch["cn"] * D],
                               in_=wsrc).then_inc(s_dma, 16)

        @block.gpsimd
        def _(g):
            g.load_library(library_config.ap_gather)
            g.wait_ge(s_in, 48)
            g.wait_ge(s_dma, 16)
            for _ in range(2):  # warmup (IRAM load) + DMA landing barrier
                g.ap_gather(
                    win_t[:, :64 * D].rearrange(
                        "p (n d) -> p n d", d=D),
                    slab_t[:].rearrange("p (n d) -> p n d", d=D),
                    idx_t[:, :4],
                    channels=128, num_elems=NELEM, d=D, num_idxs=64,
                )
            for si in range(SLOTS):
                if si > 1:
                    # win half consumed when DVE finished slot si-2
                    g.wait_ge(s_v, NCHUNK * (si - 1))
                g.ap_gather(
                    win_t[:, (si % 2) * NTOT * D:
                          ((si % 2) + 1) * NTOT * D].rearrange(
                        "p (n d) -> p n d", d=D),
                    slab_t[:].rearrange("p (n d) -> p n d", d=D),
                    idx_t[:, si * (NTOT // 16):(si + 1) * (NTOT // 16)],
                    channels=128, num_elems=NELEM, d=D, num_idxs=NTOT,
                ).then_inc(s_g, 1)

        H = D // 2

        @block.vector
        def _(v):
            for n, ch in enumerate(chunks):
                v.wait_ge(s_g, ch["si"] + 1)
                v.wait_ge(s_dma, 16 * (n + 1))
                if n > 1:
                    v.wait_ge(s_a, n - 1)  # prod buffer consumed by ACT
                if ch["cidx"] == 0 and ch["si"] > 1:
                    v.wait_ge(s_mm, ch["si"] - 1)  # red consumed by matmul
                prod = prod_ts[n % 2]
                red = red_ts[ch["si"] % 2]
                wslice = win_t[:, (ch["si"] % 2) * NTOT * D
                               + ch["xi0"] * NMAX * D:
                               (ch["si"] % 2) * NTOT * D
                               + (ch["xi0"] * NMAX + ch["cn"]) * D]
                # e=0: mul + fold + reduce on DVE
                v.tensor_mul(
                    prod[:, :ch["cn"] * D],
                    wslice,
                    wt_ts[n % 2][:, :ch["cn"] * D])
                pv = prod[:, :ch["cn"] * D].rearrange("p (x l) -> p x l", l=D)
                v.tensor_add(
                    fold_t[:, :ch["cn"] * H].rearrange(
                        "p (x h) -> p x h", h=H),
                    pv[:, :, 0:H],
                    pv[:, :, H:D],
                )
                v.tensor_reduce(
                    out=red[:, ch["xi0"]:ch["xi0"] + ch["nxi"]],
                    in_=fold_t[:, :ch["cn"] * H].rearrange(
                        "p (x l) -> p x l", l=NMAX * H),
                    axis=mybir.AxisListType.X,
                    op=mybir.AluOpType.add,
                )
                # e=1: mul only; ACT engine reduces from prod
                v.tensor_mul(
                    prod[:, :ch["cn"] * D],
                    wslice,
                    wt_ts[n % 2][:, ch["cn"] * D:2 * ch["cn"] * D])
                # drain fence: DVE in-order; this inc observes mul-e1
                v.tensor_copy(
                    vscr_t[:, :1], prod[:, :1],
                ).then_inc(s_v, 1)

        @block.tensor
        def _(t):
            for si in range(SLOTS):
                t.wait_ge(s_a, (si + 1) * NCHUNK)
                if si > 1:
                    t.wait_ge(s_od, si - 1)  # psum consumed by scalar copy
                t.matmul(psum_ts[si % 2][:], sel_t[:], red_ts[si % 2][:],
                         start=True, stop=True).then_inc(s_mm, 1)

        @block.scalar
        def _(sc):
            for n, ch in enumerate(chunks):
                sc.wait_ge(s_v, n + 1)
                if ch["cidx"] == 0 and ch["si"] > 1:
                    sc.wait_ge(s_mm, ch["si"] - 1)  # red consumed by matmul
                prod = prod_ts[n % 2]
                red = red_ts[ch["si"] % 2]
                for xi in range(ch["nxi"]):
                    col = NXI + ch["xi0"] + xi
                    sc.activation(
                        asink_t[:],
                        prod[:, xi * NMAX * D:(xi + 1) * NMAX * D],
                        mybir.ActivationFunctionType.Copy,
                        accum_out=red[:, col:col + 1],
                    )
                sc.copy(ascr_t[:, :1], red[:, NXI + ch["xi0"]:
                                           NXI + ch["xi0"] + 1]
                        ).then_inc(s_a, 1)
                if ch["cidx"] == NCHUNK - 1 and ch["si"] > 0:
                    si = ch["si"] - 1  # previous slot's psum is ready now
                    sc.wait_ge(s_mm, si + 1)
                    sc.copy(sino_t[:, si * NRAYS_G:(si + 1) * NRAYS_G],
                            psum_ts[si % 2][:])
                    sc.copy(ascr16_t[:, :1],
                            sino_t[:, si * NRAYS_G:si * NRAYS_G + 1]
                            ).then_inc(s_od, 1)
            si = SLOTS - 1
            sc.wait_ge(s_mm, si + 1)
            sc.copy(sino_t[:, si * NRAYS_G:(si + 1) * NRAYS_G],
                    psum_ts[si % 2][:])
            sc.copy(ascr16_t[:, :1],
                    sino_t[:, si * NRAYS_G:si * NRAYS_G + 1]
                    ).then_inc(s_od, 1)
            sc.wait_ge(s_od, SLOTS)
            sc.dma_start(out=out_d[:], in_=sino_t[:]).then_inc(s_od, 16)
            sc.wait_ge(s_od, 16 + SLOTS)

    mybir.codegen_inst_isa_subclasses(nc)
    _PROG_CACHE["prog"] = nc
    return nc


def kernel(image):
    image = np.asarray(image, np.float32)
    assert image.shape == (BATCH, 1, IMG_SIZE, IMG_SIZE)
    plan = _get_plan()
    nc = _build_program()

    from concourse.bass_utils import run_bass_kernel_spmd

    slabs = {rep: _build_slab(image, rep) for rep in (0, 1)}
    in_maps = []
    for ci in range(8):
        in_maps.append({
            "slab": slabs[CORE_REP[ci]],
            "idx": plan["core_idx"][ci],
            "w": plan["core_w"][ci],
            "sel": plan["sel"],
        })

    trace = bool(os.environ.get("RADON_TRACE"))
    if trace:
        _install_profhook()
    res = run_bass_kernel_spmd(nc, in_maps, list(range(8)), trace=trace)
    if trace:
        kernel.last_exec_time_ns = res.exec_time_ns

    sino = np.zeros((BATCH, 1, S, N_ANGLES), np.float32)
    for ci in range(8):
        o = res.results[ci]["out"]  # [16, SLOTS*NRAYS_G]
        for si, k in enumerate(CORE_ANGLES[ci]):
            v = (o[:, si * NRAYS_G:(si + 1) * NRAYS_G]
                 .reshape(8, 2, 2, NXI))  # [g, b, e, xi]
            # x = 16*xi + 2g + e
            full = v.transpose(1, 3, 0, 2).reshape(BATCH, NXI * 16)
            sino[:, 0, :, k] = full[:, :S]
    return sino


def _install_profhook():
    import types
    if "antenv.axon_hooks" in sys.modules:
        return
    try:
        from trn_agent_boot.trn_boot import _ntff_profile_via_ctypes
        hook = _ntff_profile_via_ctypes("/opt/axon/libaxon_pjrt.so")
    except Exception:
        hook = None
    mod = types.ModuleType("antenv.axon_hooks")
    mod._hook = hook
    mod.set_axon_ntff_profile_hook = lambda h: setattr(mod, "_hook", h)
    mod.get_axon_ntff_profile_hook = lambda: mod._hook
    sys.modules["antenv.axon_hooks"] = mod
    import antenv
    antenv.axon_hooks = mod


if __name__ == "__main__":
    img = np.load("/tmp/ref_image.npy")
    out = kernel(image=img)
    exp = np.load("/tmp/ref_expected.npy")
    err = np.linalg.norm(out - exp) / np.linalg.norm(exp)
    print("kernel rel err:", err)


# revision 57
# speedup vs baseline: 1.0722x; 1.0722x over previous
"""Radon transform (bilinear grid-sample + row-sum) on 8 TRN2 NeuronCores.

Angle wedges sharded across 8 cores (rep-pure wedges: identity frame for
|cos|>=|sin|, transposed frame otherwise). Per core, per angle: detector
rays are PAIRED (x = 16*xi + 2g + e); each pair is decomposed into 32
8-row blocks of the content region, and one GPSIMD ap_gather index per
(pair, block) fetches a 16-wide column window (hop-4 aligned, overlapping
slab storage) covering all bilinear taps of both rays in that block. The
16 channels of each Q7 core hold 8 row phases x 2 batches, so every
gathered lane is useful. Per chunk, DVE multiplies the windows by
precomputed tap weights for e=0, half-folds, and segment-reduces each ray;
the e=1 product is reduced per-ray on the Scalar engine via activation
accum_out (splitting the reduce across engines — the gather and DVE
2-port ops share the Q7 SBUF port slot, so DVE work is kept lean). A
TensorE sel-matmul sums the 128 partitions into (group, batch) sinogram
rows. All indices/weights are input-independent and precomputed on host.
"""
import math
import os
import sys
from contextlib import ExitStack

import numpy as np

sys.path.insert(0, "/opt/trn_rl_repo")

import ml_dtypes  # noqa: E402

BF16 = ml_dtypes.bfloat16

# ─── geometry constants (hardcoded for 256x256, 180 angles, batch 2) ───
N_ANGLES = 180
IMG_SIZE = 256
BATCH = 2
S = int(math.ceil(math.sqrt(2.0) * IMG_SIZE))  # 363
PB = (S - IMG_SIZE) // 2                       # 53
ROFF = 53         # slab row/col origin = content origin
HOP = 4           # window alignment granularity
D = 18            # window width (bf16 elems per gather block)
NH = 64           # hop positions per slab row
NJ = 32           # 8-row blocks covering the 256 content rows
NELEM = NJ * NH   # 2048 gather blocks per slab partition
NMAX = NJ         # block slots per ray-triple (j used directly)
NE = 3            # rays per window (x = 24*xi + 3g + e)
NXI = 16          # ray-triples per Q7 group
NRAYS_G = NE * NXI             # 48 ray columns per group (e-major)
NTOT = NXI * NMAX              # 512 indices per slot per group
SEG = NMAX * D                 # 576 elems reduced per (ray, e)
NDVE2 = 2         # e=2 rays per chunk reduced on DVE (rest on ACT)
SLOTS = 23
CHUNK_NXI = [6, 5, 5]          # ray-triples per chunk
NCHUNK = len(CHUNK_NXI)

CORE_ANGLES = [
    list(range(0, 23)), list(range(23, 46)),
    list(range(135, 158)), list(range(158, 180)),
    list(range(46, 69)), list(range(69, 91)),
    list(range(91, 113)), list(range(113, 135)),
]
CORE_REP = [0, 0, 0, 0, 1, 1, 1, 1]


def _angle_taps(k):
    """Content-region bilinear taps in rep-frame coords.

    Returns rep, xs (detector ray), j (8-row block), phi (row phase),
    cc (slab col = col-ROFF), ws (f32 weight)."""
    th = np.float32(k) * np.float32(np.pi / N_ANGLES)
    c = np.cos(th, dtype=np.float32)
    s = np.sin(th, dtype=np.float32)
    lin = np.linspace(-1.0, 1.0, S, dtype=np.float32)
    gx = c * lin[None, :] + s * lin[:, None]
    gy = -s * lin[None, :] + c * lin[:, None]
    ix = (gx + np.float32(1)) * np.float32(0.5) * np.float32(S - 1)
    iy = (gy + np.float32(1)) * np.float32(0.5) * np.float32(S - 1)
    x0 = np.floor(ix).astype(np.int64)
    y0 = np.floor(iy).astype(np.int64)
    wx = (ix - x0).astype(np.float32)
    wy = (iy - y0).astype(np.float32)
    rep = 0 if abs(c) >= abs(s) else 1
    rows_l, cols_l, ws_l, xs_l = [], [], [], []
    for dy in (0, 1):
        for dx in (0, 1):
            r = y0 + dy
            q = x0 + dx
            w = (wy if dy else 1 - wy) * (wx if dx else 1 - wx)
            m = ((r >= PB) & (r < PB + IMG_SIZE)
                 & (q >= PB) & (q < PB + IMG_SIZE) & (w != 0))
            _, xx = np.nonzero(m)
            rows_l.append(r[m])
            cols_l.append(q[m])
            ws_l.append(w[m])
            xs_l.append(xx)
    rows = np.concatenate(rows_l)
    cols = np.concatenate(cols_l)
    ws = np.concatenate(ws_l)
    xs = np.concatenate(xs_l)
    if rep:
        rows, cols = cols, rows
    j = (rows - ROFF) // 8
    phi = (rows - ROFF) % 8
    cc = cols - ROFF
    return rep, xs, j, phi, cc, ws


def _plan_angle(k):
    """Pair layout: pair p2 = x//2 (g = p2%8, xi = p2//8), e = x%2.

    Returns idx [184, NJ] int16 block ids per pair, and
    wt [2, 184, NJ, 8, D] f32 weights (e-major)."""
    rep, xs, j, phi, cc, ws = _angle_taps(k)
    t = xs // NE
    e = xs % NE
    qmin = np.full((128, NJ), 10 ** 6, np.int64)
    np.minimum.at(qmin, (t, j), cc)
    hq = np.clip(qmin // HOP, 0, 62)
    wt = np.zeros((NE, 128, NJ, 8, D), np.float32)
    kk = cc - HOP * hq[t, j]
    assert kk.min() >= 0 and kk.max() < D, (k, kk.min(), kk.max())
    np.add.at(wt, (e, t, j, phi, kk), ws)
    present = np.zeros((128, NJ), bool)
    present[t, j] = True
    idx = np.where(present, np.arange(NJ)[None, :] * NH + hq, 0)
    return rep, idx.astype(np.int16), wt


_PLAN_CACHE = {}


def _get_plan():
    if "plan" in _PLAN_CACHE:
        return _PLAN_CACHE["plan"]
    slot_w = SLOTS * NE * NTOT * D
    core_idx = []
    core_w = []
    for ci in range(8):
        idx_blob = np.zeros((128, SLOTS * (NTOT // 16)), np.int16)
        w_blob = np.zeros((64, slot_w), np.float32)
        for si, k in enumerate(CORE_ANGLES[ci]):
            rep, idx, wt = _plan_angle(k)
            assert rep == CORE_REP[ci]
            # idx[p2, j] -> group g = p2%8, n = xi*NJ + j
            ig = idx.reshape(NXI, 8, NJ).transpose(1, 0, 2).reshape(8, NTOT)
            wrap = ig.reshape(8, NTOT // 16, 16)
            for g in range(8):
                idx_blob[16 * g:16 * g + 16,
                         si * (NTOT // 16):(si + 1) * (NTOT // 16)] = wrap[g].T
            # wt[e, p2, j, phi, k] -> row 8g+phi,
            # chunk-major cols: [chunk][e][xi_local][j][k]
            wg = (wt.reshape(NE, NXI, 8, NJ, 8, D)
                  .transpose(2, 4, 0, 1, 3, 5))  # [g, phi, e, xi, j, k]
            base = si * NE * NTOT * D
            xi0 = 0
            for nxi in CHUNK_NXI:
                sz = NE * nxi * NJ * D
                blockw = (wg[:, :, :, xi0:xi0 + nxi]
                          .reshape(64, sz))
                w_blob[:, base:base + sz] = blockw
                base += sz
                xi0 += nxi
        core_idx.append(idx_blob)
        core_w.append(w_blob.astype(BF16))
    sel = np.zeros((128, 16), np.float32)
    for p in range(128):
        sel[p, 2 * (p // 16) + (p % 2)] = 1.0
    plan = dict(core_idx=core_idx, core_w=core_w, sel=sel)
    _PLAN_CACHE["plan"] = plan
    return plan


def _build_slab(image, rep):
    """[128, NELEM*D] bf16: channel p%16 = 2*phi+b holds hop-4 overlapping
    windows of content rows ROFF+8j+phi (replicated across the 8 groups)."""
    fr = np.zeros((BATCH, S, S), np.float32)
    fr[:, PB:PB + IMG_SIZE, PB:PB + IMG_SIZE] = image[:, 0]
    if rep:
        fr = np.ascontiguousarray(np.transpose(fr, (0, 2, 1)))
    out = np.zeros((16, NELEM * D), np.float32)
    span = HOP * (NH - 1) + D  # 268 cols
    for phi in range(8):
        rows = fr[:, ROFF + phi: ROFF + phi + 8 * NJ: 8, ROFF:ROFF + span]
        win = np.lib.stride_tricks.sliding_window_view(rows, D, axis=2)
        win = win[:, :, ::HOP, :]  # [B, NJ, NH, D]
        assert win.shape == (BATCH, NJ, NH, D)
        for b in range(BATCH):
            out[2 * phi + b] = win[b].reshape(-1)
    out16 = out.astype(BF16)
    return np.ascontiguousarray(np.broadcast_to(
        out16[None], (8, 16, NELEM * D)).reshape(128, NELEM * D))


_PROG_CACHE = {}


def _build_program():
    if "prog" in _PROG_CACHE:
        return _PROG_CACHE["prog"]
    import concourse.bass as bass
    import concourse.mybir as mybir
    from concourse import library_config

    nc = bass.Bass()
    slab_d = nc.declare_dram_parameter("slab", [128, NELEM * D],
                                       mybir.dt.bfloat16, isOutput=False)
    idx_d = nc.declare_dram_parameter("idx", [128, SLOTS * (NTOT // 16)],
                                      mybir.dt.int16, isOutput=False)
    w_d = nc.declare_dram_parameter("w", [64, SLOTS * NE * NTOT * D],
                                    mybir.dt.bfloat16, isOutput=False)
    sel_d = nc.declare_dram_parameter("sel", [128, 16], mybir.dt.float32,
                                      isOutput=False)
    out_d = nc.declare_dram_parameter("out", [16, SLOTS * NRAYS_G],
                                      mybir.dt.float32, isOutput=True)

    ctx = ExitStack()
    with ctx:
        slab_t = ctx.enter_context(
            nc.sbuf_tensor([128, NELEM * D], mybir.dt.bfloat16))
        idx_t = ctx.enter_context(
            nc.sbuf_tensor([128, SLOTS * (NTOT // 16)], mybir.dt.int16))
        maxw = max(CHUNK_NXI) * NJ  # windows per chunk
        wt_ts = [ctx.enter_context(
            nc.sbuf_tensor(f"wt{i}", [128, NE * maxw * D], mybir.dt.bfloat16))
            for i in range(2)]
        # one gather per slot; two slot-parity halves for pipelining
        win_t = ctx.enter_context(
            nc.sbuf_tensor("win", [128, 2 * NTOT * D], mybir.dt.bfloat16))
        # prod: [e0/e1 scratch | e2] per buffer
        prod_ts = [ctx.enter_context(
            nc.sbuf_tensor(f"prod{i}", [128, 2 * maxw * D],
                           mybir.dt.bfloat16))
            for i in range(2)]
        fold_t = ctx.enter_context(
            nc.sbuf_tensor([128, maxw * (D // 2)], mybir.dt.bfloat16))
        asink_t = ctx.enter_context(
            nc.sbuf_tensor([128, NMAX * D], mybir.dt.bfloat16))
        red_ts = [ctx.enter_context(
            nc.sbuf_tensor(f"red{i}", [128, NRAYS_G], mybir.dt.float32))
            for i in range(2)]
        sel_t = ctx.enter_context(nc.sbuf_tensor([128, 16], mybir.dt.float32))
        vscr_t = ctx.enter_context(nc.sbuf_tensor([128, 2], mybir.dt.float32))
        ascr_t = ctx.enter_context(nc.sbuf_tensor([128, 2], mybir.dt.float32))
        ascr16_t = ctx.enter_context(nc.sbuf_tensor([16, 2], mybir.dt.float32))
        sino_t = ctx.enter_context(
            nc.sbuf_tensor("sino", [16, SLOTS * NRAYS_G], mybir.dt.float32))
        psum_ts = [ctx.enter_context(
            nc.psum_tensor(f"ps{i}", [16, NRAYS_G], mybir.dt.float32))
            for i in range(2)]
        s_in = ctx.enter_context(nc.semaphore("s_in"))
        s_dma = ctx.enter_context(nc.semaphore("s_dma"))
        s_g = ctx.enter_context(nc.semaphore("s_g"))
        s_v = ctx.enter_context(nc.semaphore("s_v"))
        s_a = ctx.enter_context(nc.semaphore("s_a"))
        s_mm = ctx.enter_context(nc.semaphore("s_mm"))
        s_od = ctx.enter_context(nc.semaphore("s_od"))
        block = ctx.enter_context(nc.Block())

        # chunk schedule: (slot, cidx, xi0, nxi, cn, idx col offset, w offset)
        chunks = []
        for si in range(SLOTS):
            xi0 = 0
            ow = si * NE * NTOT * D
            for cidx, nxi in enumerate(CHUNK_NXI):
                cn = nxi * NMAX
                chunks.append(dict(
                    si=si, cidx=cidx, xi0=xi0, nxi=nxi, cn=cn, ow=ow))
                ow += NE * cn * D
                xi0 += nxi

        @block.sync
        def _(sync):
            sync.dma_start(out=slab_t[:], in_=slab_d[:]).then_inc(s_in, 16)
            sync.dma_start(out=idx_t[:], in_=idx_d[:]).then_inc(s_in, 16)
            sync.dma_start(out=sel_t[:], in_=sel_d[:]).then_inc(s_in, 16)
            for n, ch in enumerate(chunks):
                if n > 1:
                    sync.wait_ge(s_v, n - 1)  # wt buffer consumed
                wsrc = (w_d[:, ch["ow"]:ch["ow"] + NE * ch["cn"] * D]
                        .unsqueeze(1)
                        .broadcast_to([64, 2, NE * ch["cn"] * D]))
                sync.dma_start(out=wt_ts[n % 2][:, :NE * ch["cn"] * D],
                               in_=wsrc).then_inc(s_dma, 16)

        @block.gpsimd
        def _(g):
            g.load_library(library_config.ap_gather)
            g.wait_ge(s_in, 48)
            g.wait_ge(s_dma, 16)
            for _ in range(2):  # warmup (IRAM load) + DMA landing barrier
                g.ap_gather(
                    win_t[:, :64 * D].rearrange(
                        "p (n d) -> p n d", d=D),
                    slab_t[:].rearrange("p (n d) -> p n d", d=D),
                    idx_t[:, :4],
                    channels=128, num_elems=NELEM, d=D, num_idxs=64,
                )
            for si in range(SLOTS):
                if si > 1:
                    # win half consumed when DVE finished slot si-2
                    g.wait_ge(s_v, NCHUNK * (si - 1))
                g.ap_gather(
                    win_t[:, (si % 2) * NTOT * D:
                          ((si % 2) + 1) * NTOT * D].rearrange(
                        "p (n d) -> p n d", d=D),
                    slab_t[:].rearrange("p (n d) -> p n d", d=D),
                    idx_t[:, si * (NTOT // 16):(si + 1) * (NTOT // 16)],
                    channels=128, num_elems=NELEM, d=D, num_idxs=NTOT,
                ).then_inc(s_g, 1)

        H = D // 2

        @block.vector
        def _(v):
            for n, ch in enumerate(chunks):
                v.wait_ge(s_g, ch["si"] + 1)
                v.wait_ge(s_dma, 16 * (n + 1))
                if n > 1:
                    v.wait_ge(s_a, n - 1)  # prod buffer consumed by ACT
                if ch["cidx"] == 0 and ch["si"] > 1:
                    v.wait_ge(s_mm, ch["si"] - 1)  # red consumed by matmul
                prod = prod_ts[n % 2]
                red = red_ts[ch["si"] % 2]
                wslice = win_t[:, (ch["si"] % 2) * NTOT * D
                               + ch["xi0"] * NMAX * D:
                               (ch["si"] % 2) * NTOT * D
                               + (ch["xi0"] * NMAX + ch["cn"]) * D]
                cnD = ch["cn"] * D
                # e=0: mul + fold + reduce on DVE
                v.tensor_mul(
                    prod[:, :cnD],
                    wslice,
                    wt_ts[n % 2][:, :cnD])
                pv = prod[:, :cnD].rearrange("p (x l) -> p x l", l=D)
                v.tensor_add(
                    fold_t[:, :ch["cn"] * H].rearrange(
                        "p (x h) -> p x h", h=H),
                    pv[:, :, 0:H],
                    pv[:, :, H:D],
                )
                v.tensor_reduce(
                    out=red[:, ch["xi0"]:ch["xi0"] + ch["nxi"]],
                    in_=fold_t[:, :ch["cn"] * H].rearrange(
                        "p (x l) -> p x l", l=NMAX * H),
                    axis=mybir.AxisListType.X,
                    op=mybir.AluOpType.add,
                )
                # e=1: mul only (ACT reduces); e=2: mul + DVE tail-reduce
                v.tensor_mul(
                    prod[:, :cnD],
                    wslice,
                    wt_ts[n % 2][:, cnD:2 * cnD])
                v.tensor_mul(
                    prod[:, cnD:2 * cnD],
                    wslice,
                    wt_ts[n % 2][:, 2 * cnD:3 * cnD])
                na = ch["nxi"] - NDVE2  # e=2 rays on ACT
                v.tensor_reduce(
                    out=red[:, 2 * NXI + ch["xi0"] + na:
                            2 * NXI + ch["xi0"] + ch["nxi"]],
                    in_=prod[:, cnD + na * SEG:
                             cnD + ch["nxi"] * SEG].rearrange(
                        "p (x l) -> p x l", l=SEG),
                    axis=mybir.AxisListType.X,
                    op=mybir.AluOpType.add,
                )
                # drain fence: DVE in-order; this inc observes all muls
                v.tensor_copy(
                    vscr_t[:, :1], prod[:, :1],
                ).then_inc(s_v, 1)

        @block.tensor
        def _(t):
            for si in range(SLOTS):
                t.wait_ge(s_a, (si + 1) * NCHUNK)
                if si > 1:
                    t.wait_ge(s_od, si - 1)  # psum consumed by scalar copy
                t.matmul(psum_ts[si % 2][:], sel_t[:], red_ts[si % 2][:],
                         start=True, stop=True).then_inc(s_mm, 1)

        @block.scalar
        def _(sc):
            for n, ch in enumerate(chunks):
                sc.wait_ge(s_v, n + 1)
                if ch["cidx"] == 0 and ch["si"] > 1:
                    sc.wait_ge(s_mm, ch["si"] - 1)  # red consumed by matmul
                prod = prod_ts[n % 2]
                red = red_ts[ch["si"] % 2]
                cnD = ch["cn"] * D
                for xi in range(ch["nxi"]):  # e=1 rays
                    col = NXI + ch["xi0"] + xi
                    sc.activation(
                        asink_t[:],
                        prod[:, xi * SEG:(xi + 1) * SEG],
                        mybir.ActivationFunctionType.Copy,
                        accum_out=red[:, col:col + 1],
                    )
                for xi in range(ch["nxi"] - NDVE2):  # e=2 head rays
                    col = 2 * NXI + ch["xi0"] + xi
                    sc.activation(
                        asink_t[:],
                        prod[:, cnD + xi * SEG:cnD + (xi + 1) * SEG],
                        mybir.ActivationFunctionType.Copy,
                        accum_out=red[:, col:col + 1],
                    )
                sc.copy(ascr_t[:, :1], red[:, NXI + ch["xi0"]:
                                           NXI + ch["xi0"] + 1]
                        ).then_inc(s_a, 1)
                if ch["cidx"] == NCHUNK - 1 and ch["si"] > 0:
                    si = ch["si"] - 1  # previous slot's psum is ready now
                    sc.wait_ge(s_mm, si + 1)
                    sc.copy(sino_t[:, si * NRAYS_G:(si + 1) * NRAYS_G],
                            psum_ts[si % 2][:])
                    sc.copy(ascr16_t[:, :1],
                            sino_t[:, si * NRAYS_G:si * NRAYS_G + 1]
                            ).then_inc(s_od, 1)
            si = SLOTS - 1
            sc.wait_ge(s_mm, si + 1)
            sc.copy(sino_t[:, si * NRAYS_G:(si + 1) * NRAYS_G],
                    psum_ts[si % 2][:])
            sc.copy(ascr16_t[:, :1],
                    sino_t[:, si * NRAYS_G:si * NRAYS_G + 1]
                    ).then_inc(s_od, 1)
            sc.wait_ge(s_od, SLOTS)
            sc.dma_start(out=out_d[:], in_=sino_t[:]).then_inc(s_od, 16)
            sc.wait_ge(s_od, 16 + SLOTS)

    mybir.codegen_inst_isa_subclasses(nc)
    _PROG_CACHE["prog"] = nc
    return nc


def kernel(image):
    image = np.asarray(image, np.float32)
    assert image.shape == (BATCH, 1, IMG_SIZE, IMG_SIZE)
    plan = _get_plan()
    nc = _build_program()

    from concourse.bass_utils import run_bass_kernel_spmd

    slabs = {rep: _build_slab(image, rep) for rep in (0, 1)}
    in_maps = []
    for ci in range(8):
        in_maps.append({
            "slab": slabs[CORE_REP[ci]],
            "idx": plan["core_idx"][ci],
            "w": plan["core_w"][ci],
            "sel": plan["sel"],
        })

    trace = bool(os.environ.get("RADON_TRACE"))
    if trace:
        _install_profhook()
    res = run_bass_kernel_spmd(nc, in_maps, list(range(8)), trace=trace)
    if trace:
        kernel.last_exec_time_ns = res.exec_time_ns

    sino = np.zeros((BATCH, 1, S, N_ANGLES), np.float32)
    for ci in range(8):
        o = res.results[ci]["out"]  # [16, SLOTS*NRAYS_G]
        for si, k in enumerate(CORE_ANGLES[ci]):
            v = (o[:, si * NRAYS_G:(si + 1) * NRAYS_G]
                 .reshape(8, 2, NE, NXI))  # [g, b, e, xi]
            # x = 24*xi + 3g + e
            full = v.transpose(1, 3, 0, 2).reshape(BATCH, NXI * 24)
            sino[:, 0, :, k] = full[:, :S]
    return sino


def _install_profhook():
    import types
    if "antenv.axon_hooks" in sys.modules:
        return
    try:
        from trn_agent_boot.trn_boot import _ntff_profile_via_ctypes
        hook = _ntff_profile_via_ctypes("/opt/axon/libaxon_pjrt.so")
    except Exception:
        hook = None
    mod = types.ModuleType("antenv.axon_hooks")
    mod._hook = hook
    mod.set_axon_ntff_profile_hook = lambda h: setattr(mod, "_hook", h)
    mod.get_axon_ntff_profile_hook = lambda: mod._hook
    sys.modules["antenv.axon_hooks"] = mod
    import antenv
    antenv.axon_hooks = mod


if __name__ == "__main__":
    img = np.load("/tmp/ref_image.npy")
    out = kernel(image=img)
    exp = np.load("/tmp/ref_expected.npy")
    err = np.linalg.norm(out - exp) / np.linalg.norm(exp)
    print("kernel rel err:", err)
